# revision 24
# baseline (speedup 1.0000x reference)
"""Multi-head self-attention (GroupNorm -> qkv -> attention -> proj) on 8 trn2 cores.

Sharding: each core owns (batch b = core//4, query-chunk q = core%4 of 1024
pixels). GroupNorm and K/V are computed redundantly per core (cheap); queries
and the attention matrix are sharded by (batch, query-range), so no
collectives are needed. Per-core inputs are column-rolled so every core's
query range is columns [0:1024] of its own x (softmax over keys is
permutation-invariant, so rolling pixels does not change results).

Device pipeline per core:
  GroupNorm(8, 256) on x (chunked DMA + bn_stats interleaved so stats start
  while x still streams in; group-combine via two tiny selector matmuls).
  q = Wq hn[:, :1024], k = Wk hn, both channels-on-partitions;
  vT = hn^T Wv^T + bv directly in [pixels, channels] layout (bv folded into
  v: softmax weights sum to 1, so o = sum a*(v+bv) = o'/Z + bv exactly).
  Attention, flash-style without max-subtraction (scores ~N(0,1), exp safe):
  per (head-pair granule, q-chunk 512, k-chunk 128): S^T row-tiled into PSUM
  [128k, 2x512]; exp split ACT (native Exp LUT, ~3/4 of granules) and DVE
  (custom (cubic)^64 two-pass op, ~1/4); AV+Z (fused ones-columns, M=64)
  accumulate o'/Z in a persistent 2-bank PSUM tile. AV emission lags 5
  granules behind exp so the PE always has deferred work (keeps PE dense for
  the HAM clock governor) and exp never waits on the PE.
  o = o'/Z (bv already folded), then out = Wp o + bp -> [256, 1024] fp32.
Matmul inputs are bf16 (full-speed PE), accumulation fp32 in PSUM;
GroupNorm statistics are computed in fp32.
"""

import ml_dtypes
import numpy as np

BF16 = ml_dtypes.bfloat16

C = 256
N = 4096
NQ = 1024
NH = 8
HD = 32
G = 8
EPS = 1e-5
P = 128
QC = 512
NKC = N // P  # 32 k-chunks
SCALE = HD ** -0.5
NCORES = 8

# pipeline tuning
AV_LAG = 5        # granules the AV matmuls trail the exp stream by
EST_BUFS = 8      # est (post-exp) SBUF tiles in flight


def _dve_granule(gi):
    # Interleaved so ACT and DVE exp run concurrently (bunched assignment
    # serializes the engines); last granules all-ACT so the DVE is free for
    # the normalize at qc end.
    return gi % 4 == 1 and gi < 60

_CACHE = {}


def _build_program():
    import concourse.bass as bass  # noqa: F401
    import concourse.tile as tile
    from concourse import bacc, mybir
    from concourse import dve_ops as dv
    from concourse.dve_spec import C0, C1, C2, One, Spec, Src0, sq
    from concourse.dve_ops import DveOp

    def reg_dve(name, spec):
        for op in dv.OPS:
            if op.name == name:
                return op
        op = DveOp(name, spec, subdim=False, uops_sha={})
        dv.OPS.append(op)
        dv.CUSTOM_DVE_SPECS[name] = spec
        dv._SUB_OPCODE_FOR_NAME[name] = dv._CUSTOM_DVE_ROW_BASE + len(dv.OPS) - 1
        for ver in ("v3", "v4"):
            try:
                op.compile(ver)
            except ValueError as e:
                op.uops_sha[ver] = str(e).split(f"{ver}: ")[1].split(" ")[0]
                op.compile(ver)
        return op

    # exp(x*SCALE) = (q(x))^64 with q a cubic fit of exp(x*SCALE/64),
    # constant term pinned to 1. op1 computes q(x)^4 (8 ALU stages), op2
    # raises to the 16th power. Validated on HW: max rel err 1.1e-4 in f32.
    E4 = reg_dve("ANT_EXP64_P4",
                 Spec(body=sq(sq(((C0 * Src0 + C1) * Src0 + C2) * Src0 + One))))
    S16 = reg_dve("ANT_SQ4", Spec(body=sq(sq(sq(sq(Src0))))))
    EC3 = 3.4064999134215023e-09
    EC2 = 3.8173198326396545e-06
    EC1 = 0.0027622298856143134

    F32 = mybir.dt.float32
    BF16 = mybir.dt.bfloat16
    AF = mybir.ActivationFunctionType
    ALU = mybir.AluOpType

    nc = bacc.Bacc("TRN2", target_bir_lowering=False)

    # Constants are packed into three blobs so startup needs 3 DMA issues
    # instead of ~23 (the Sync engine serializes DMA_DIRECT2D issues at
    # ~650ns each, which was the real startup cost).
    x_d = nc.declare_dram_parameter("x", [C, N], BF16, isOutput=False)
    wblob_d = nc.declare_dram_parameter("wblob", [P, 8 * C], BF16, isOutput=False)
    cblob_d = nc.declare_dram_parameter("cblob", [P, 10 + C], F32, isOutput=False)
    bblob_d = nc.declare_dram_parameter("bblob", [P, 4 + P], BF16, isOutput=False)
    out_d = nc.declare_dram_parameter("out", [C, NQ], BF16, isOutput=True)

    with tile.TileContext(nc) as tc:
        const = tc.alloc_tile_pool(name="const", bufs=1)
        big = tc.alloc_tile_pool(name="big", bufs=1)
        work = tc.alloc_tile_pool(name="work", bufs=2)
        expp = tc.alloc_tile_pool(name="expp", bufs=EST_BUFS)
        psp = tc.alloc_tile_pool(name="psp", bufs=3, space="PSUM")
        ozp = tc.alloc_tile_pool(name="ozp", bufs=1, space="PSUM")

        # ---- constant loads (3 blob DMAs; everything else is a view) ----
        # issue order: small blobs (gate GroupNorm) before the big weight
        # blob (only needed once qkv starts ~25us in); x chunks in between
        wblob = const.tile([P, 8 * C], BF16, name="wblob", tag="wblob")
        cblob = const.tile([P, 10 + C], F32, name="cblob", tag="cblob")
        bblob = const.tile([P, 4 + P], BF16, name="bblob", tag="bblob")
        nc.sync.dma_start(out=bblob, in_=bblob_d[:, :])
        nc.sync.dma_start(out=cblob, in_=cblob_d[:, :])
        wq_sb = [wblob[:, (0 + i) * C:(1 + i) * C] for i in range(2)]
        wk_sb = [wblob[:, (2 + i) * C:(3 + i) * C] for i in range(2)]
        wv_sb = [wblob[:, (4 + i) * C:(5 + i) * C] for i in range(2)]
        wp_sb = [wblob[:, (6 + i) * C:(7 + i) * C] for i in range(2)]
        bq_sb = [cblob[:, i:i + 1] for i in range(2)]
        bk_sb = [cblob[:, 2 + i:3 + i] for i in range(2)]
        bp_sb = [cblob[:, 4 + i:5 + i] for i in range(2)]
        gnw_sb = [cblob[:, 6 + i:7 + i] for i in range(2)]
        gnb_sb = [cblob[:, 8 + i:9 + i] for i in range(2)]
        bvT_sb = cblob[:, 10:10 + C]
        gsel_sb = bblob[:, 0:4]
        gselT_sb = bblob[0:4, 4:4 + P]
        eps_sb = const.tile([4, 1], F32, name="eps", tag="eps")
        nc.vector.memset(eps_sb, EPS)

        # PE warm-up during the x DMA so the HAM activity window sees a busy
        # stretch before the qkv burst.
        wps = psp.tile([P, C], F32, name="wps", tag="ps")
        for _ in range(12):
            nc.tensor.matmul(out=wps, lhsT=wq_sb[0][:, 0:P], rhs=wk_sb[0],
                             start=True, stop=True)

        # vT3[p, kc, head, 0:32] = v^T channels (+ bv); cols 32:64 = 1.0
        # (fused Z accumulation: the M=64 AV matmul emits o' rows 0-31 and Z
        # broadcast over rows 32-63). Whole tile memset to 1.0 here, during
        # the x-DMA wait while the DVE is otherwise idle; v-planes are
        # overwritten by emit_vt.
        vT_sb = big.tile([P, NKC, NH, 64], BF16, name="vt", tag="vt")
        nc.vector.memset(vT_sb, 1.0)

        # ---- x load (bf16 halves DMA volume), chunked so bn_stats overlaps
        # the DMA ----
        NCH = 4
        CW = N // NCH  # 1024
        xch = [[big.tile([P, CW], BF16, name=f"x{t}_{j}", tag=f"x{t}_{j}")
                for j in range(NCH)] for t in range(2)]
        for t in range(2):
            for j in range(NCH):
                nc.sync.dma_start(
                    out=xch[t][j],
                    in_=x_d[t * P:(t + 1) * P, j * CW:(j + 1) * CW])
        nc.sync.dma_start(out=wblob, in_=wblob_d[:, :])

        hn_sb = [big.tile([P, N], BF16, name=f"hn{t}", tag=f"hn{t}") for t in range(2)]

        # ---- GroupNorm (stats via bn_stats per chunk, group-combine through
        # two tiny selector matmuls), normalized bf16 output to hn_sb ----
        for t in range(2):
            stats = work.tile([P, 2 * NCH, 6], F32, name="gnstats", tag="gnstats")
            for j in range(NCH):
                xv = xch[t][j].rearrange("p (a b) -> p a b", b=512)
                for i in range(2):
                    nc.vector.bn_stats(out=stats[:, 2 * j + i, :], in_=xv[:, i, :])
            mv = work.tile([P, 2], F32, name="gnmv", tag="gnmv")
            nc.vector.bn_aggr(out=mv, in_=stats)
            # st2 = (mean_c, E[x^2]_c), bf16 for the selector matmul
            st2 = work.tile([P, 2], BF16, name="gnst2", tag="gnst2")
            e2f = work.tile([P, 1], F32, name="gne2", tag="gne2")
            nc.vector.tensor_mul(out=e2f, in0=mv[:, 0:1], in1=mv[:, 0:1])
            nc.vector.tensor_add(out=e2f, in0=e2f, in1=mv[:, 1:2])
            nc.vector.tensor_copy(out=st2[:, 0:1], in_=mv[:, 0:1])
            nc.vector.tensor_copy(out=st2[:, 1:2], in_=e2f)
            # group combine: [4,2] = (mean_g, E2_g) via selector matmul (gsel = 1/32)
            gp = psp.tile([P, 2], F32, name="psgn", tag="ps")
            nc.tensor.matmul(out=gp[0:4, 0:2], lhsT=gsel_sb, rhs=st2,
                             start=True, stop=True)
            vg = work.tile([4, 1], F32, name="gnvg", tag="gnvg")
            gm = work.tile([4, 2], F32, name="gngm", tag="gngm")
            g2 = work.tile([4, 2], BF16, name="gng2", tag="gng2")
            nc.vector.tensor_copy(out=gm, in_=gp[0:4, 0:2])
            nc.vector.tensor_copy(out=g2[:, 0:1], in_=gm[:, 0:1])
            nc.vector.tensor_mul(out=vg, in0=gm[:, 0:1], in1=gm[:, 0:1])
            nc.vector.tensor_sub(out=vg, in0=gm[:, 1:2], in1=vg)
            nc.scalar.activation(out=vg, in_=vg, func=AF.Sqrt, bias=eps_sb)
            nc.vector.reciprocal(out=vg, in_=vg)
            nc.vector.tensor_copy(out=g2[:, 1:2], in_=vg)
            # broadcast to channels: [128,2] = (mean_c', rstd_c')
            bc = psp.tile([P, 2], F32, name="psgn2", tag="ps")
            nc.tensor.matmul(out=bc[:, 0:2], lhsT=gselT_sb, rhs=g2,
                             start=True, stop=True)
            s_t = work.tile([P, 1], F32, name="gns", tag="gns")
            t_t = work.tile([P, 1], F32, name="gnt", tag="gnt")
            nc.vector.tensor_mul(out=s_t, in0=bc[:, 1:2], in1=gnw_sb[t])
            nc.vector.tensor_mul(out=t_t, in0=bc[:, 0:1], in1=s_t)
            nc.vector.tensor_sub(out=t_t, in0=gnb_sb[t], in1=t_t)
            for j in range(NCH):
                nc.vector.tensor_scalar(
                    out=hn_sb[t][:, j * CW:(j + 1) * CW], in0=xch[t][j],
                    scalar1=s_t, scalar2=t_t, op0=ALU.mult, op1=ALU.add)

        # ---- qkv + attention ----
        q_sb = [big.tile([P, NQ], BF16, name=f"q{i}", tag=f"q{i}") for i in range(2)]
        k_sb = [big.tile([P, N], BF16, name=f"k{i}", tag=f"k{i}") for i in range(2)]

        def emit_q_chunk(hg, qcc):
            ps = psp.tile([P, QC], F32, name="psq", tag="ps")
            for cc in range(2):
                nc.tensor.matmul(
                    out=ps,
                    lhsT=wq_sb[cc][:, hg * P:(hg + 1) * P],
                    rhs=hn_sb[cc][:, qcc * QC:(qcc + 1) * QC],
                    start=(cc == 0), stop=(cc == 1))
            nc.vector.tensor_scalar(
                out=q_sb[hg][:, qcc * QC:(qcc + 1) * QC], in0=ps,
                scalar1=bq_sb[hg], scalar2=None, op0=ALU.add)

        def emit_k_chunk(hg, ncc):
            ps = psp.tile([P, QC], F32, name="psk", tag="ps")
            for cc in range(2):
                nc.tensor.matmul(
                    out=ps,
                    lhsT=wk_sb[cc][:, hg * P:(hg + 1) * P],
                    rhs=hn_sb[cc][:, ncc * QC:(ncc + 1) * QC],
                    start=(cc == 0), stop=(cc == 1))
            nc.vector.tensor_scalar(
                out=k_sb[hg][:, ncc * QC:(ncc + 1) * QC], in0=ps,
                scalar1=bk_sb[hg], scalar2=None, op0=ALU.add)

        def emit_vt(ncc):
            ps = psp.tile([P, C], F32, name="psv", tag="ps")
            for cc in range(2):
                nc.tensor.matmul(
                    out=ps,
                    lhsT=hn_sb[cc][:, ncc * P:(ncc + 1) * P],
                    rhs=wv_sb[cc],
                    start=(cc == 0), stop=(cc == 1))
            # v + bv (bias folded into v; exact since softmax weights sum to 1)
            nc.vector.tensor_add(
                out=vT_sb[:, ncc, :, 0:32],
                in0=ps.rearrange("p (h d) -> p h d", h=NH),
                in1=bvT_sb.rearrange("p (h d) -> p h d", h=NH))

        # Only q chunk 0 and k chunk 0 are needed for the first S granules;
        # the rest stream into the kc loop (k chunk ncc is consumed from
        # kc = 4*ncc, emitted at kc = 4*ncc - 3).
        emit_q_chunk(0, 0)
        emit_k_chunk(0, 0)

        # ---- attention ----
        # Granule = (head pair pr, k-chunk kc): S^T [128k, 2x512q] in a PSUM
        # tile (bufs=3); persistent 2-bank oz accumulates o' and (fused ones
        # columns) Z across all 32 k-chunks. Exp split between ACT (Exp LUT)
        # and DVE (custom (cubic)^64). AV lags AV_LAG granules so the PE
        # always has deferred work and never gates the exp engines.
        o_flat = [big.tile([P, NQ], BF16, name=f"of{i}", tag=f"of{i}")
                  for i in range(2)]
        out_sb = [big.tile([P, NQ], BF16, name=f"out{i}", tag=f"out{i}")
                  for i in range(2)]

        def emit_proj(ncc, cs=0, cw=QC):
            # proj for query-chunk ncc (cols cs:cs+cw within it) needs
            # o_flat[:, ...] of BOTH channel halves -> runnable right after
            # (hg=1, qc=ncc)'s normalize, overlapping remaining work.
            for rc in range(2):
                ps = psp.tile([P, QC], F32, name="psp2", tag="ps")
                for cc in range(2):
                    nc.tensor.matmul(
                        out=ps[:, 0:cw],
                        lhsT=wp_sb[cc][:, rc * P:(rc + 1) * P],
                        rhs=o_flat[cc][:, ncc * QC + cs:ncc * QC + cs + cw],
                        start=(cc == 0), stop=(cc == 1))
                nc.vector.tensor_scalar(
                    out=out_sb[rc][:, ncc * QC + cs:ncc * QC + cs + cw],
                    in0=ps[:, 0:cw],
                    scalar1=bp_sb[rc], scalar2=None, op0=ALU.add)
                nc.sync.dma_start(
                    out=out_d[rc * P:(rc + 1) * P, ncc * QC + cs:ncc * QC + cs + cw],
                    in_=out_sb[rc][:, ncc * QC + cs:ncc * QC + cs + cw])

        for hg in range(2):
            for qc in range(NQ // QC):
                qoff = qc * QC
                oz = ozp.tile([P, 2 * QC], F32, name="oz", tag="oz")
                pending = []

                def do_av(item, oz=oz, hg=hg):
                    # h = 2*pr + h' -> oz quadrant: rows 64*(h%2), bank h//2.
                    est_, kc_, pr_ = item
                    for h2 in range(2):
                        h = 2 * pr_ + h2
                        hh = hg * 4 + h
                        nc.tensor.matmul(
                            out=oz[64 * (h % 2):64 * (h % 2) + 64,
                                   QC * (h // 2):QC * (h // 2) + QC],
                            lhsT=vT_sb[:, kc_, hh, :],
                            rhs=est_[:, h2 * QC:(h2 + 1) * QC],
                            start=(kc_ == 0), stop=(kc_ == NKC - 1),
                            tile_position=(0, 64 * (h % 2)))

                gi = 0
                for kc in range(NKC):
                    if hg == 0 and qc == 0:
                        emit_vt(kc)
                        if kc % 4 == 1 and kc < 29:
                            emit_k_chunk(0, (kc + 3) // 4)
                        elif kc == 2:
                            emit_q_chunk(0, 1)
                    if hg == 0 and qc == 1:
                        # hg1 prep interleaved so there is no serial bubble
                        # between head-groups
                        if kc == 2:
                            emit_q_chunk(1, 0)
                        elif kc == 4:
                            emit_q_chunk(1, 1)
                        elif 6 <= kc <= 27 and (kc - 6) % 3 == 0:
                            emit_k_chunk(1, (kc - 6) // 3)
                    for pr in range(2):
                        S = psp.tile([P, 2 * QC], F32, name="ps", tag="ps")
                        for h in range(2):
                            row = 64 * pr + 32 * h
                            nc.tensor.matmul(
                                out=S[:, h * QC:(h + 1) * QC],
                                lhsT=k_sb[hg][row:row + 32, kc * P:(kc + 1) * P],
                                rhs=q_sb[hg][row:row + 32, qoff:qoff + QC],
                                start=True, stop=True, tile_position=(row, 0))
                        est = expp.tile([P, 2 * QC], BF16, name="est",
                                        tag="est", bufs=EST_BUFS)
                        if _dve_granule(gi):
                            etmp = expp.tile([P, 2 * QC], F32, name="etmp",
                                             tag="etmp", bufs=2)
                            nc.vector._custom_dve(E4, out=etmp, in0=S[:],
                                                  s0=EC3, s1=EC2, imm2=EC1)
                            nc.vector._custom_dve(S16, out=est, in0=etmp)
                        else:
                            nc.scalar.activation(out=est, in_=S[:],
                                                 func=AF.Exp, scale=SCALE)
                        gi += 1
                        pending.append((est, kc, pr))
                        # shorter lag in the last phase: the drain is pure
                        # tail there, nothing left to overlap
                        lag = 2 if (hg, qc) == (1, 1) else AV_LAG
                        if len(pending) > lag:
                            do_av(pending.pop(0))
                for item in pending:
                    do_av(item)
                # normalize: unpack oz quadrants via DMA, then o = o'/Z
                # (bv already folded into v). ozc copy on ACT (balances the
                # exp load); zB DMAs issue before oP so the reciprocal
                # starts earlier. The very last phase runs in column halves
                # to pipeline normalize -> proj -> out-DMA in the tail.
                ozc = work.tile([P, 2 * QC], F32, name="ozc", tag="ozc")
                nc.scalar.copy(out=ozc, in_=oz)
                oP = work.tile([P, QC], F32, name="oP", tag="oP")
                zB = work.tile([P, QC], F32, name="zB", tag="zB")
                zr = work.tile([P, QC], F32, name="zr", tag="zr")
                halves = 2 if (hg, qc) == (1, 1) else 1
                hw = QC // halves
                for hf in range(halves):
                    cs = hf * hw
                    for h in range(4):
                        r0 = 64 * (h % 2)
                        c0 = QC * (h // 2)
                        nc.sync.dma_start(
                            out=zB[32 * h:32 * h + 32, cs:cs + hw],
                            in_=ozc[r0 + 32:r0 + 64, c0 + cs:c0 + cs + hw])
                    nc.vector.reciprocal(out=zr[:, cs:cs + hw],
                                         in_=zB[:, cs:cs + hw])
                    for h in range(4):
                        r0 = 64 * (h % 2)
                        c0 = QC * (h // 2)
                        nc.sync.dma_start(
                            out=oP[32 * h:32 * h + 32, cs:cs + hw],
                            in_=ozc[r0:r0 + 32, c0 + cs:c0 + cs + hw])
                    nc.vector.tensor_mul(
                        out=o_flat[hg][:, qoff + cs:qoff + cs + hw],
                        in0=oP[:, cs:cs + hw], in1=zr[:, cs:cs + hw])
                    if hg == 1:
                        emit_proj(qc, cs, hw)

        for pool in (ozp, psp, expp, work, big, const):
            pool.release()

    nc.compile()
    return nc


def kernel(x, gn_weight, gn_bias, w_qkv, b_qkv, w_proj, b_proj):
    from concourse.bass_utils import run_bass_kernel_spmd

    x = np.asarray(x, dtype=np.float32)
    B = x.shape[0]
    xf = x.reshape(B, C, N)

    wqT = np.ascontiguousarray(np.asarray(w_qkv, np.float32)[0:C, :].T).astype(BF16)
    wkT = np.ascontiguousarray(np.asarray(w_qkv, np.float32)[C:2 * C, :].T).astype(BF16)
    wvT = np.ascontiguousarray(np.asarray(w_qkv, np.float32)[2 * C:3 * C, :].T).astype(BF16)
    wpT = np.ascontiguousarray(np.asarray(w_proj, np.float32).T).astype(BF16)
    wblob = np.concatenate(
        [wqT[0:P], wqT[P:C], wkT[0:P], wkT[P:C],
         wvT[0:P], wvT[P:C], wpT[0:P], wpT[P:C]], axis=1)
    wblob = np.ascontiguousarray(wblob)

    bq = np.asarray(b_qkv, np.float32)[0:C]
    bk = np.asarray(b_qkv, np.float32)[C:2 * C]
    bv = np.asarray(b_qkv, np.float32)[2 * C:3 * C]
    bp = np.asarray(b_proj, np.float32)
    gnw = np.asarray(gn_weight, np.float32)
    gnb = np.asarray(gn_bias, np.float32)
    cblob = np.empty((P, 10 + C), np.float32)
    for i in range(2):
        sl = slice(i * P, (i + 1) * P)
        cblob[:, 0 + i] = bq[sl]
        cblob[:, 2 + i] = bk[sl]
        cblob[:, 4 + i] = bp[sl]
        cblob[:, 6 + i] = gnw[sl]
        cblob[:, 8 + i] = gnb[sl]
    cblob[:, 10:10 + C] = bv[None, :]

    bblob = np.zeros((P, 4 + P), BF16)
    for c in range(P):
        bblob[c, c // HD] = 1.0 / HD        # gsel
        bblob[c // HD, 4 + c] = 1.0         # gselT (rows 0-3 used)

    shared = dict(wblob=wblob, cblob=cblob, bblob=bblob)
    xb = xf.astype(BF16)
    in_maps = []
    for core in range(NCORES):
        b = core // 4
        roff = (core % 4) * NQ
        xr = np.roll(xb[b], -roff, axis=1)
        m = dict(shared)
        m["x"] = np.ascontiguousarray(xr)
        in_maps.append(m)

    if "nc" not in _CACHE:
        _CACHE["nc"] = _build_program()
    nc = _CACHE["nc"]

    res = run_bass_kernel_spmd(nc, in_maps, list(range(NCORES)))
    _CACHE["last_result"] = res
    out = np.empty((B, C, N), np.float32)
    for core in range(NCORES):
        b = core // 4
        roff = (core % 4) * NQ
        out[b][:, roff:roff + NQ] = np.asarray(res.results[core]["out"]).astype(np.float32)
    return out.reshape(B, C, 64, 64)
